# revision 89
# baseline (speedup 1.0000x reference)
"""Trainium2 Bass kernel for nn_Attention_49598282334528.

Dense transformer attention block: fused QKV projection + RoPE + causal
GQA flash-style attention + output projection, for
  x: [2, 2048, 2048], H=16 q heads, KV=4 kv heads, head_dim=128.

Sharding (8 NeuronCores): data-parallel over batch (2) x tensor-parallel
over kv-head groups (4).  Core c handles batch c//4, kv-group c%4 (4 q
heads + 1 kv head).  Each core computes a full-width partial of the
output projection (row-parallel Wo); the host sums the 4 partials per
batch and stacks batches.

Design (software-pipelined, low-precision matmul operands):
  - Matmul operands are bf16 (q/k path, projections, Wo) or fp16 (the
    exp'd P matrix and v); both run 1 PE cycle/row at ANY output width
    (fp32r pays 4x under 256), accumulation stays fp32 in PSUM.  This
    halves DMA and SBUF and licenses full causal trimming.
  - One program-wide pool set; per-block pipeline proj(b) -> attn(b),
    with block b-1's outproj emitted one row-tile per attention head
    boundary (and block 1's q projections at attn(0) boundaries) as
    fine-grained PE filler for the exp stalls.  PSUM: proj/outproj
    share a 2-bank ring, scores 4 banks, oT/sm 2 banks.
  - Block 0's projection runs di-outer across all six outputs
    (borrowing 6 psum banks, all idle before attention starts) so each
    weight/x DMA chunk is consumed as it lands instead of stalling on
    full-weight arrival.
  - Projection outputs (q0..q3, k, v) each evict psum->sbuf via the
    scalar engine; RoPE runs on DVE in 4 ops/output on SBUF bf16 (2x
    DVE mode) using host-prepared [cos;cos] / [sin;-sin] tables laid
    out so both inputs of each mul share a base partition.
  - Attention is emitted skew-2: scores(ki) / exp(ki-1) / PV(ki-2);
    exp maps psum->fp16 with a constant -2 bias (cancels in the
    softmax ratio, keeps fp16 sums in range).  Diagonal tiles are
    masked by a DVE multiply with a 0/1 triangular tile — no PE mask
    matmuls.
  - Softmax denominators: exp tiles are accumulated on the DVE (fp16,
    2x) and reduced by ONE ones-matmul per (block, head) — instead of
    a PE row-sum matmul per ki tile (that costs as much as PV).
    The denominator/normalize tail is deferred into the next head's
    score stream to hide its latency.
  - Output projection partials evict psum->sbuf on DVE and DMA out as
    fp16 (host upcasts and sums the 4 row-parallel partials).
  - DMAs are split per ~0.5MB chunk and emitted in need-order so the
    first matmul starts ~1us in instead of waiting for all weights.

Do NOT strip PE same-engine semaphore waits here: with bf16 matmuls
(separate Ldweights+Matmult) that reordering license NaN'd on hardware
and wedged the device, while CoreSim stayed clean.
"""

import sys

if "/opt/trn_rl_repo" not in sys.path:
    sys.path.insert(0, "/opt/trn_rl_repo")

import numpy as np

B, S, D = 2, 2048, 2048
H, KV, HD = 16, 4, 128
G = 4                # kv groups == cores per batch
QPH = H // KV        # q heads per group = 4
EQ = QPH * HD        # per-core q width = 512
NCORES = 8
P = 128
ABLK = 512           # seq block (both proj and attention sq block)
NA = S // ABLK       # 4
ND = D // P          # 16 contraction chunks
SCALE = 1.0 / float(np.sqrt(HD))

_CACHE = {}


def _build_program():
    import concourse.bass as bass
    import concourse.tile as tile
    from concourse import bacc, mybir

    f32 = mybir.dt.float32
    bf16 = mybir.dt.bfloat16
    fp16 = mybir.dt.float16
    EXP = mybir.ActivationFunctionType.Exp
    COPY = mybir.ActivationFunctionType.Copy
    EXPB = -2.0  # constant exp bias; cancels in softmax ratio, keeps the
    # fp16 P-matrix and its fp16-accumulated row sums well inside range

    nc = bacc.Bacc("TRN2", target_bir_lowering=False, debug=False)

    # host-prearranged inputs (see _prep_inputs for layouts)
    xtb = nc.dram_tensor("xtb", [NA, P, ND, ABLK], bf16, kind="ExternalInput").ap()
    wq = nc.dram_tensor("wq", [P, ND, EQ], bf16, kind="ExternalInput").ap()
    wk = nc.dram_tensor("wk", [P, ND, HD], bf16, kind="ExternalInput").ap()
    wv = nc.dram_tensor("wv", [P, ND, HD], bf16, kind="ExternalInput").ap()
    wo = nc.dram_tensor("wo", [P, QPH, D], bf16, kind="ExternalInput").ap()
    cc = nc.dram_tensor("cc", [P, S], bf16, kind="ExternalInput").ap()      # [cos;cos]
    sspm = nc.dram_tensor("sspm", [P, S], bf16, kind="ExternalInput").ap()  # [-sin;sin]
    ones_d = nc.dram_tensor("ones_d", [P, P], fp16, kind="ExternalInput").ap()
    ident_d = nc.dram_tensor("ident_d", [P, P], fp16, kind="ExternalInput").ap()
    mask01_d = nc.dram_tensor("mask01_d", [P, P], fp16, kind="ExternalInput").ap()
    outp = nc.dram_tensor("outp", [S, D], fp16, kind="ExternalOutput").ap()

    with tile.TileContext(nc) as tc:
        import contextlib

        with contextlib.ExitStack() as stack:
            const = stack.enter_context(tc.tile_pool(name="const", bufs=1))
            persist = stack.enter_context(tc.tile_pool(name="persist", bufs=1))
            wpool = stack.enter_context(tc.tile_pool(name="wproj", bufs=1))
            xtp = stack.enter_context(tc.tile_pool(name="xtp", bufs=4))
            # psum pools: pj(2, shared with outproj) + st(4, also hosts
            # sm/vtp) + oT/sm(2) = 8 banks
            pj = stack.enter_context(
                tc.tile_pool(name="pjps", bufs=2, space="PSUM"))
            stvt = stack.enter_context(
                tc.tile_pool(name="stps", bufs=4, space="PSUM"))
            ac = stack.enter_context(
                tc.tile_pool(name="acps", bufs=1, space="PSUM"))
            # sbuf working pools
            pevt = stack.enter_context(tc.tile_pool(name="pevt", bufs=4))
            rtmp = stack.enter_context(tc.tile_pool(name="rtmp", bufs=3))
            stsb = stack.enter_context(tc.tile_pool(name="stsb", bufs=12))
            nrm = stack.enter_context(tc.tile_pool(name="nrm", bufs=4))
            osg = stack.enter_context(tc.tile_pool(name="osg", bufs=6))

            # ---- persistent tensors ----
            expb_sb = const.tile([P, 1], f32)
            nc.gpsimd.memset(expb_sb[:], EXPB)
            cc_sb = const.tile([P, S], bf16)
            ss_sb = const.tile([P, S], bf16)
            ones_sb = const.tile([P, P], fp16)
            ident_sb = const.tile([P, P], fp16)
            mask01_sb = const.tile([P, P], fp16)

            wq_sb = wpool.tile([P, ND, EQ], bf16)
            wk_sb = wpool.tile([P, ND, HD], bf16)
            wv_sb = wpool.tile([P, ND, HD], bf16)
            wo_sb = wpool.tile([P, QPH, D], bf16)

            qT_blks = [persist.tile([P, QPH, ABLK], bf16, name=f"qTb{b}")
                       for b in range(NA)]
            kT_blks = [persist.tile([P, ABLK], bf16, name=f"kTb{b}")
                       for b in range(NA)]
            v_blks = [persist.tile([P, ABLK // P, HD], fp16, name=f"vb{b}")
                      for b in range(NA)]
            oT_blks = [persist.tile([P, QPH, ABLK], bf16, name=f"oTb{b}")
                       for b in range(NA)]

            # ---- DMA emission, need-order ----
            # first: weights for q0 + x block 0, then consts, then the rest
            def dma(dst, src):
                nc.sync.dma_start(out=dst, in_=src)

            xt_tiles = [None] * NA

            def load_x_block(blk):
                t = xtp.tile([P, ND, ABLK], bf16, tag="xt", name=f"xt{blk}")
                for g in range(4):
                    dma(t[:, 4 * g : 4 * g + 4, :],
                        xtb[blk, :, 4 * g : 4 * g + 4, :])
                xt_tiles[blk] = t

            # need-order: alternate wq / x-block-0 chunks (finest first so the
            # first matmul starts ~1us in), then rope tables, then k/v
            # weights, then the rest.
            xt0 = xtp.tile([P, ND, ABLK], bf16, tag="xt", name="xt0")
            xt_tiles[0] = xt0
            # block-0 chunks interleaved in di-outer consumption order
            for di in range(2):
                dma(wq_sb[:, di, :], wq[:, di, :])
                dma(xt0[:, di, :], xtb[0, :, di, :])
                dma(wk_sb[:, di, :], wk[:, di, :])
                dma(wv_sb[:, di, :], wv[:, di, :])
            for di in range(2, 4):
                dma(wq_sb[:, di, :], wq[:, di, :])
                dma(xt0[:, di, :], xtb[0, :, di, :])
            dma(wk_sb[:, 2:4, :], wk[:, 2:4, :])
            dma(wv_sb[:, 2:4, :], wv[:, 2:4, :])
            for g in range(1, 4):
                sl = slice(4 * g, 4 * g + 4)
                dma(wq_sb[:, sl, :], wq[:, sl, :])
                dma(xt0[:, sl, :], xtb[0, :, sl, :])
                dma(wk_sb[:, sl, :], wk[:, sl, :])
                dma(wv_sb[:, sl, :], wv[:, sl, :])
            dma(cc_sb[:], cc[:])
            dma(ss_sb[:], sspm[:])
            dma(ident_sb[:], ident_d[:])
            dma(ones_sb[:], ones_d[:])
            dma(mask01_sb[:], mask01_d[:])
            load_x_block(1)
            for h in range(QPH):
                dma(wo_sb[:, h, :], wo[:, h, :])
            load_x_block(2)
            load_x_block(3)

            # ---- per-block pipeline ----
            def rope(pe_sb, s0, dst):
                """dst = rope(pe_sb) using [cos;cos] / [-sin;sin] tables."""
                HH = HD // 2
                cp = rtmp.tile([P, ABLK], bf16, tag="cp", name="cp")
                tm = rtmp.tile([P, ABLK], bf16, tag="tm", name="tm")
                nc.vector.tensor_mul(cp[:], pe_sb[:], cc_sb[:, s0 : s0 + ABLK])
                # ss_sb rows 64:128 hold -sin, rows 0:64 hold +sin, so each
                # mul's two SBUF inputs share a base partition (hw constraint)
                nc.vector.tensor_mul(
                    tm[0:HH, :], pe_sb[HH:P, :], ss_sb[HH:P, s0 : s0 + ABLK])
                nc.vector.tensor_mul(
                    tm[HH:P, :], pe_sb[0:HH, :], ss_sb[0:HH, s0 : s0 + ABLK])
                nc.vector.tensor_add(dst, cp[:], tm[:])

            def _proj_w(oi, di):
                if oi < QPH:
                    return wq_sb[:, di, oi * HD : (oi + 1) * HD]
                if oi == QPH:
                    return wk_sb[:, di, :]
                return wv_sb[:, di, :]

            def _proj_consume(blk, oi, pp):
                s0 = blk * ABLK
                if oi == 5:
                    pe = pevt.tile([P, ABLK], fp16, tag="pev", name="pev")
                else:
                    pe = pevt.tile([P, ABLK], bf16, tag="pe", name="pe")
                nc.scalar.activation(pe[:], pp[:], COPY)
                if oi < QPH:
                    rope(pe, s0, qT_blks[blk][:, oi, :])
                elif oi == QPH:
                    rope(pe, s0, kT_blks[blk][:])
                else:
                    for j in range(ABLK // P):
                        vp = stvt.tile([P, P], fp16, tag="st", name="vtp")
                        nc.tensor.transpose(
                            vp[:], pe[:, j * P : (j + 1) * P], ident_sb[:])
                        nc.scalar.activation(
                            v_blks[blk][:, j, :], vp[:], COPY)

            def attn_head(blk, h, pending):
                """Scores/exp/PV pipeline for one head.  The denominator+
                normalize tail is deferred (returned via `pending`) so the
                next head's score stream covers its DVE latency."""
                s0 = blk * ABLK
                n_sk = (blk + 1) * (ABLK // P)
                qT = qT_blks[blk][:, h, :]
                oT_ps = ac.tile([P, ABLK], f32, tag="oT", name="oTps")

                st_ps_l = [None] * n_sk
                st_t_l = [None] * n_sk
                acc = nrm.tile([P, ABLK], fp16, tag="acc", name="acc")

                def emit_scores(ki):
                    lead = max(ki * P - s0, 0)
                    sp = stvt.tile([P, ABLK], f32, tag="st", name="stps")
                    nc.tensor.matmul(
                        sp[:, lead:],
                        kT_blks[ki // 4][:, (ki % 4) * P : (ki % 4 + 1) * P],
                        qT[:, lead:],
                        start=True, stop=True,
                    )
                    st_ps_l[ki] = sp

                def emit_exp(ki):
                    lead = max(ki * P - s0, 0)
                    stt = stsb.tile([P, ABLK], fp16, tag="stt", name="stt")
                    nc.scalar.activation(
                        stt[:, lead:], st_ps_l[ki][:, lead:], EXP,
                        scale=SCALE, bias=expb_sb[:],
                    )
                    if ki * P >= s0:  # diagonal tile: zero above-diag
                        nc.vector.tensor_mul(
                            stt[:, lead : lead + P],
                            stt[:, lead : lead + P],
                            mask01_sb[:],
                        )
                    # running row-sum accumulation on DVE (fp16, 2x mode)
                    if ki == 0:
                        nc.vector.tensor_scalar_mul(acc[:], stt[:], 1.0)
                    else:
                        nc.vector.tensor_add(
                            acc[:, lead:], acc[:, lead:], stt[:, lead:])
                    st_t_l[ki] = stt

                def emit_pv(ki):
                    lead = max(ki * P - s0, 0)
                    first = ki == 0
                    last = ki == n_sk - 1
                    nc.tensor.matmul(
                        oT_ps[:, lead:],
                        v_blks[ki // 4][:, ki % 4, :],
                        st_t_l[ki][:, lead:],
                        start=first, stop=last,
                    )
                    st_t_l[ki] = None
                    st_ps_l[ki] = None

                # software-pipelined emission, skew 2; flush the previous
                # head's tail once our first scores are in flight
                for ki in range(n_sk):
                    emit_scores(ki)
                    if ki == 1 and pending[0] is not None:
                        pending[0]()
                        pending[0] = None
                    if ki >= 1:
                        emit_exp(ki - 1)
                    if ki >= 2:
                        emit_pv(ki - 2)
                emit_exp(n_sk - 1)
                emit_pv(n_sk - 2)
                emit_pv(n_sk - 1)

                def tail():
                    # row-sum matmul over the accumulated exp tiles; halves
                    # pipeline the ones->recip->mul chain
                    sm_ps = ac.tile([P, ABLK], f32, tag="sm", name="smps")
                    rc = nrm.tile([P, ABLK], f32, tag="rc", name="rc")
                    HB = ABLK // 2
                    for s in range(2):
                        sl = slice(s * HB, (s + 1) * HB)
                        nc.tensor.matmul(
                            sm_ps[:, sl], ones_sb[:], acc[:, sl],
                            start=True, stop=True)
                        nc.vector.reciprocal(rc[:, sl], sm_ps[:, sl])
                        nc.vector.tensor_mul(
                            oT_blks[blk][:, h, sl], oT_ps[:, sl], rc[:, sl])

                pending[0] = tail

            def outproj(blk, t_list=None, cb_list=None):
                # outproj rides the proj psum ring: by emission time the
                # current block's projections are consumed, so both rotate
                # through the 2 banks double-buffered
                for t in (range(ABLK // P) if t_list is None else t_list):
                    row = blk * (ABLK // P) + t
                    for cb in (range(D // 512) if cb_list is None
                               else cb_list):
                        op_ps = pj.tile([P, ABLK], f32, tag="pp",
                                        name="opps")
                        for h in range(QPH):
                            nc.tensor.matmul(
                                op_ps[:],
                                oT_blks[blk][:, h, t * P : (t + 1) * P],
                                wo_sb[:, h, cb * 512 : (cb + 1) * 512],
                                start=(h == 0), stop=(h == QPH - 1),
                            )
                        ob = osg.tile([P, 512], fp16, tag="ob", name="ob")
                        nc.vector.tensor_scalar_mul(ob[:], op_ps[:], 1.0)
                        nc.sync.dma_start(
                            out=outp[row * P : (row + 1) * P,
                                     cb * 512 : (cb + 1) * 512],
                            in_=ob[:],
                        )

            def proj_single(blk, oi):
                xt_t = xt_tiles[blk]
                pp = pj.tile([P, ABLK], f32, tag="pp", name="pp")
                for di in range(ND):
                    nc.tensor.matmul(
                        pp[:], _proj_w(oi, di), xt_t[:, di, :],
                        start=(di == 0), stop=(di == ND - 1),
                    )
                _proj_consume(blk, oi, pp)

            def proj_block0_di_outer():
                """Block 0 is DMA-throttled: iterate di outer across ALL six
                outputs so each weight/x chunk is consumed as it lands.
                Borrows 6 psum banks across the three pools (all idle before
                attention starts)."""
                xt_t = xt_tiles[0]
                ps = [
                    pj.tile([P, ABLK], f32, tag="pp", name="pp"),
                    pj.tile([P, ABLK], f32, tag="pp", name="pp"),
                    stvt.tile([P, ABLK], f32, tag="st", name="stps"),
                    stvt.tile([P, ABLK], f32, tag="st", name="stps"),
                    stvt.tile([P, ABLK], f32, tag="st", name="stps"),
                    ac.tile([P, ABLK], f32, tag="oT", name="oTps"),
                ]
                for di in range(ND):
                    st_, sp_ = di == 0, di == ND - 1
                    for oi in range(6):
                        nc.tensor.matmul(
                            ps[oi][:], _proj_w(oi, di), xt_t[:, di, :],
                            start=st_, stop=sp_)
                # consume in attn-need order: q0, k, v first
                for oi in (0, 4, 5, 1, 2, 3):
                    _proj_consume(0, oi, ps[oi])

            pending = [None]
            for blk in range(NA):
                if blk == 0:
                    proj_block0_di_outer()
                elif blk == 1:
                    # q0..q3 were already emitted at attn(0) head boundaries
                    proj_single(blk, 4)
                    proj_single(blk, 5)
                else:
                    for oi in range(6):
                        proj_single(blk, oi)
                for h in range(QPH):
                    attn_head(blk, h, pending)
                    if blk == 0:
                        # block 1's q projections double as attn(0) filler
                        proj_single(1, h)
                    else:
                        # outproj delayed one block, one row-tile per head
                        # boundary: fine-grained filler for exp stalls
                        outproj(blk - 1, t_list=[h])
            if pending[0] is not None:
                # the last head normalizes inline (quarter-pipelined
                # denominators), so pending is normally already empty
                pending[0]()
            outproj(NA - 1)

    # NOTE: the old _strip_pe_self_waits hack is intentionally NOT applied:
    # it was only safe for fp32r self-loading (S3_LW) matmuls.  bf16 matmuls
    # lower to separate Ldweights+Matmult, and stripping same-engine waits
    # there lets the backend reorder PE instructions (observed as NaN output
    # and a wedged device).
    nc.finalize()
    return nc


def _bf16(a):
    import ml_dtypes

    return np.asarray(a, np.float32).astype(ml_dtypes.bfloat16)


def _prep_inputs(x, freqs_cos, freqs_sin, Wq, Wk, Wv, Wo):
    """Build the 8 per-core input maps (pure layout work, no arithmetic)."""
    perm = np.concatenate([np.arange(0, HD, 2), np.arange(1, HD, 2)])

    cosT = freqs_cos.T.astype(np.float32)  # [64, S]
    sinT = freqs_sin.T.astype(np.float32)
    cc = _bf16(np.concatenate([cosT, cosT], axis=0))          # [128, S]
    sspm = _bf16(np.concatenate([sinT, -sinT], axis=0))       # [128, S]
    ones = np.ones((P, P), np.float16)
    ident = np.eye(P, dtype=np.float16)
    # mask01[p, j] = 1 where j >= p (keep), else 0
    mask01 = np.triu(np.ones((P, P), np.float16))

    # xtb[blk, p, di, s] = x[b].T[di*128+p, blk*512+s]
    xtbs = []
    for b in range(B):
        xT = np.ascontiguousarray(x[b].T.astype(np.float32))  # [D, S]
        t = xT.reshape(ND, P, NA, ABLK).transpose(2, 1, 0, 3)
        xtbs.append(_bf16(np.ascontiguousarray(t)))

    wqs, wks, wvs, wos = [], [], [], []
    for g in range(G):
        wq_g = Wq[:, g * EQ : (g + 1) * EQ].reshape(D, QPH, HD)[:, :, perm]
        wq_g = wq_g.reshape(D, EQ).reshape(ND, P, EQ).transpose(1, 0, 2)
        wqs.append(_bf16(np.ascontiguousarray(wq_g)))         # [128, 16, 512]
        wk_g = Wk[:, g * HD : (g + 1) * HD][:, perm]
        wk_g = wk_g.reshape(ND, P, HD).transpose(1, 0, 2)
        wks.append(_bf16(np.ascontiguousarray(wk_g)))         # [128, 16, 128]
        wv_g = Wv[:, g * HD : (g + 1) * HD]
        wv_g = wv_g.reshape(ND, P, HD).transpose(1, 0, 2)
        wvs.append(_bf16(np.ascontiguousarray(wv_g)))
        wo_g = Wo[g * EQ : (g + 1) * EQ, :]                   # [512, D]
        wo_g = wo_g.reshape(QPH, P, D).transpose(1, 0, 2)
        wos.append(_bf16(np.ascontiguousarray(wo_g)))         # [128, 4, 2048]

    in_maps = []
    for c in range(NCORES):
        b, g = divmod(c, G)
        in_maps.append(
            dict(xtb=xtbs[b], wq=wqs[g], wk=wks[g], wv=wvs[g], wo=wos[g],
                 cc=cc, sspm=sspm, ones_d=ones, ident_d=ident,
                 mask01_d=mask01)
        )
    return in_maps


LAST_RESULTS = None


def kernel(**inputs) -> np.ndarray:
    global LAST_RESULTS
    x = np.asarray(inputs["x"], np.float32)
    in_maps = _prep_inputs(
        x,
        np.asarray(inputs["freqs_cos"], np.float32),
        np.asarray(inputs["freqs_sin"], np.float32),
        np.asarray(inputs["Wq"], np.float32),
        np.asarray(inputs["Wk"], np.float32),
        np.asarray(inputs["Wv"], np.float32),
        np.asarray(inputs["Wo"], np.float32),
    )

    if "nc" not in _CACHE:
        _CACHE["nc"] = _build_program()
    nc = _CACHE["nc"]

    from concourse import bass_utils

    res = bass_utils.run_bass_kernel_spmd(nc, in_maps, list(range(NCORES)))
    LAST_RESULTS = res

    out = np.empty((B, S, D), np.float32)
    for b in range(B):
        acc = res.results[4 * b]["outp"].astype(np.float32)
        for g in range(1, G):
            acc = acc + res.results[4 * b + g]["outp"]
        out[b] = acc
    return out
